# revision 15
# baseline (speedup 1.0000x reference)
"""Single-head attention (B=4, S=4096, D=A=1024, fp32 I/O) on 8 TRN2 NeuronCores.

Sharding: core c handles batch b=c//2, sequence-half h=c%2 (2048 rows).
Each core projects Q, K^T and V for its own half only; core pairs exchange
K^T/V halves with chunked AllGathers (overlapped with projection compute), so
nothing is computed twice.  Attention then runs flash-style per 512-query
block against the full gathered sequence.

Device layout is transpose-free: host passes x[b]^T slices and pre-transposed
weights, all pre-tiled so that every DRAM->SBUF load is contiguous per
partition (128-row tiles stacked along the free axis).  Q^T stays resident in
SBUF between the projection and attention phases; K^T goes through DRAM only
for the pair exchange, laid out so phase-2 streaming loads are contiguous.
Scores are computed transposed ([k,q]); softmax normalization is deferred to
the output projection epilogue (exp without max subtraction is safe here:
scores are O(5)).  Softmax denominators accumulate on the vector engine
(elementwise over k-tiles) with a single ones-matmul for the final partition
reduction.  Matmul compute in bf16, accumulation fp32.  k-tiles are
enumerated in gather order everywhere, which keeps scores, exp, sums and ctx
consistent without any index remapping.
"""

import numpy as np
import ml_dtypes

import concourse.bass as bass
import concourse.tile as tile
from concourse import mybir
from concourse.bass_utils import run_bass_kernel_spmd

BF = mybir.dt.bfloat16
F32 = mybir.dt.float32
AF = mybir.ActivationFunctionType

B, S, DIM, A = 4, 4096, 1024, 1024
SQ = S // 2          # rows handled per core (query rows and local K/V rows)
NC = DIM // 128      # d chunks
NA = A // 128        # a tiles
NK = S // 128        # k tiles (global)
QB = 512             # q block width
NQB = SQ // QB
SB = SQ // 512       # 512-col blocks of the local sequence half
SCALE = 1.0 / np.sqrt(np.float32(A))

N_CORES = 8
PAIRS = [[0, 1], [2, 3], [4, 5], [6, 7]]

LAST_RESULT = None   # BassKernelResults of the most recent run (for test.py)


def _split_multiwaits(nc):
    """This walrus build rejects instructions carrying more than one sem wait
    (and Drains carrying any); hoist extra waits into single-wait NoOps
    preceding the instruction on the same engine."""
    for f in nc.m.functions:
        for bb in f.blocks:
            new_insts = []
            for inst in bb.instructions:
                si = inst.sync_info
                if si is not None and si.on_wait:
                    keep = 0 if isinstance(inst, mybir.InstDrain) else 1
                    if len(si.on_wait) > keep:
                        waits = list(si.on_wait)
                        hoist, rest = waits[: len(waits) - keep], waits[len(waits) - keep :]
                        for w in hoist:
                            nop = mybir.InstNoOp(
                                name=nc.get_next_instruction_name(),
                                sync_info=mybir.SyncInfo(on_wait=[w], on_update=[]),
                                bass_nofuse=True,
                                engine=inst.engine,
                            )
                            nc.register_instruction(nop)
                            new_insts.append(nop)
                        si.on_wait.clear()
                        si.on_wait.extend(rest)
                new_insts.append(inst)
            bb.instructions[:] = new_insts


def _build():
    nc = bass.Bass()

    # all inputs pre-tiled on host: [sb][128][c][cols] so device loads are
    # contiguous per partition
    xTq = nc.declare_dram_parameter("xTq", [SB, 128, NC, 512], BF, isOutput=False)
    WqT = nc.declare_dram_parameter("WqT", [128, NC, A], BF, isOutput=False)
    WkT = nc.declare_dram_parameter("WkT", [128, NC, A], BF, isOutput=False)
    WvT = nc.declare_dram_parameter("WvT", [128, NC, A], BF, isOutput=False)
    WoT = nc.declare_dram_parameter("WoT", [128, NC, DIM], BF, isOutput=False)
    bqc = nc.declare_dram_parameter("bqc", [128, NA], F32, isOutput=False)
    bkc = nc.declare_dram_parameter("bkc", [128, NA], F32, isOutput=False)
    bvb = nc.declare_dram_parameter("bvb", [128, A], BF, isOutput=False)
    bob = nc.declare_dram_parameter("bob", [128, DIM], BF, isOutput=False)
    out = nc.declare_dram_parameter("out", [SQ, DIM], F32, isOutput=True)

    with tile.TileContext(nc) as tc:
        with (
            tc.tile_pool(name="dram", bufs=1, space="DRAM") as dram,
            tc.tile_pool(name="singles", bufs=1) as singles,
        ):
            # collective buffers, [sb][p][am][k] so both the ko stores and
            # the phase-2 streaming loads are contiguous per partition.
            # K^T and V each exchange in ONE AllGather: while a collective
            # transfers, regular DMA queues starve, so fewer/bigger windows
            # beat a fine-grained pipeline.
            kt_in = dram.tile([SB, 128, NA, 512], BF, name="kt_in", tag="kti")
            kt_out = dram.tile([2, SB, 128, NA, 512], BF, name="kt_out", tag="kto")
            v_in = dram.tile([SQ, A], BF, name="v_in", tag="vi")
            v_out = dram.tile([2, SQ, A], BF, name="v_out", tag="vo")

            warm_in = dram.tile([1, 128], BF, name="warm_in")
            warm_out = dram.tile([2, 1, 128], BF, name="warm_out")

            v_sb = singles.tile([128, NK, A], BF)        # V resident, 8.4 MB
            qt_sb = singles.tile([128, NA, SQ], BF)      # Q^T resident, 4.2 MB
            bqc_sb = singles.tile([128, NA], F32)
            bkc_sb = singles.tile([128, NA], F32)
            ones_k = singles.tile([128, 1], F32)         # sums matmul lhsT
            ones_1 = singles.tile([1, 1], F32)           # row->partition matmul rhs

            # phase-2 streaming pool allocated BEFORE the phase-1 pools so
            # its SBUF addresses are disjoint from phase-1 tiles -> its
            # prefetch DMAs carry no WAR dependency on phase-1 compute
            ksp = tc.tile_pool(name="p2k", bufs=3)
            p2k = ksp.__enter__()

            # ---------------- Phase 1: projections + K/V exchange ----------
            with (
                tc.tile_pool(name="p1w", bufs=1) as p1w,
                tc.tile_pool(name="p1x", bufs=1) as p1x,
                tc.tile_pool(name="p1b", bufs=1) as p1b,
                tc.tile_pool(name="p1ko", bufs=10) as p1ko,
                tc.tile_pool(name="p1vo", bufs=5) as p1vo,
                tc.tile_pool(name="p1pk", bufs=2, space="PSUM") as p1pk,
                tc.tile_pool(name="p1pv", bufs=2, space="PSUM") as p1pv,
            ):
                wk = p1w.tile([128, NC, A], BF, tag="wkq")
                wv = p1w.tile([128, NC, A], BF, tag="wv")
                bvb_sb = p1b.tile([128, A], BF)
                # all of x^T stays resident through phase 1 so no PE input
                # depends on DMA while the collectives are saturating HBM
                xs_all = p1x.tile([128, SB, NC, 512], BF)

                # wake the collectives firmware immediately (the first
                # collective otherwise pays ~25us of startup latency in the
                # middle of the K/V exchange chain); staged through an
                # internal DRAM tile since collectives can't read I/O tensors
                nc.gpsimd.dma_start(out=warm_in[:], in_=xTq[0, 0:1, 0, 0:128])
                nc.gpsimd.collective_compute(
                    "AllGather",
                    mybir.AluOpType.bypass,
                    replica_groups=PAIRS,
                    ins=[warm_in[:].opt()],
                    outs=[warm_out[:].opt()],
                )
                # minimal DMA before the first matmul (split loads so dc=0
                # matmuls start early); every weight load is queued ahead of
                # the ko/vo stores so no load ever trails the store chain
                nc.sync.dma_start(out=wk[:, 0:2, :], in_=WkT[:, 0:2, :])
                nc.sync.dma_start(out=xs_all[:, 0, 0:2, :], in_=xTq[0, :, 0:2, :])
                nc.scalar.dma_start(out=wk[:, 2:8, :], in_=WkT[:, 2:8, :])
                nc.scalar.dma_start(out=bkc_sb[:], in_=bkc[:])
                nc.sync.dma_start(out=xs_all[:, 0, 2:8, :], in_=xTq[0, :, 2:8, :])
                for sb in range(1, SB):
                    nc.sync.dma_start(out=xs_all[:, sb, :, :], in_=xTq[sb])
                nc.sync.dma_start(out=wv[:], in_=WvT[:])
                nc.scalar.dma_start(out=bvb_sb[:], in_=bvb[:])
                nc.scalar.dma_start(out=bqc_sb[:], in_=bqc[:])

                # K^T projection for the local half, then one pair AllGather
                for sb in range(SB):
                    for am in range(NA):
                        pk = p1pk.tile([128, 512], F32)
                        for dc in range(NC):
                            nc.tensor.matmul(
                                pk[:],
                                lhsT=wk[:, dc, am * 128 : (am + 1) * 128],
                                rhs=xs_all[:, sb, dc, :],
                                start=(dc == 0),
                                stop=(dc == NC - 1),
                            )
                        ko = p1ko.tile([128, 512], BF)
                        nc.scalar.activation(
                            ko[:], pk[:], AF.Identity, bias=bkc_sb[:, am : am + 1]
                        )
                        nc.sync.dma_start(out=kt_in[sb, :, am, :], in_=ko[:])
                nc.gpsimd.collective_compute(
                    "AllGather",
                    mybir.AluOpType.bypass,
                    replica_groups=PAIRS,
                    ins=[kt_in[:].opt()],
                    outs=[kt_out[:].opt()],
                )

                # wq reuses wk's SBUF slot; its load wires in after the last
                # K matmul retires
                wq = p1w.tile([128, NC, A], BF, tag="wkq")
                nc.sync.dma_start(out=wq[:], in_=WqT[:])

                # V projection for the local half, then one pair AllGather
                for sb in range(SB):
                    for st in range(4):
                        pv = p1pv.tile([128, 1024], F32)
                        for half in range(2):
                            for dc in range(NC):
                                nc.tensor.matmul(
                                    pv[:, half * 512 : (half + 1) * 512],
                                    lhsT=xs_all[:, sb, dc, st * 128 : (st + 1) * 128],
                                    rhs=wv[:, dc, half * 512 : (half + 1) * 512],
                                    start=(dc == 0),
                                    stop=(dc == NC - 1),
                                )
                        vo = p1vo.tile([128, 1024], BF, tag="vo")
                        nc.vector.tensor_add(vo[:], pv[:], bvb_sb[:])
                        nc.scalar.dma_start(
                            out=v_in[(sb * 4 + st) * 128 : (sb * 4 + st + 1) * 128, :],
                            in_=vo[:],
                        )
                nc.gpsimd.collective_compute(
                    "AllGather",
                    mybir.AluOpType.bypass,
                    replica_groups=PAIRS,
                    ins=[v_in[:].opt()],
                    outs=[v_out[:].opt()],
                )

                # gathered V -> resident SBUF, k enumerated in gather order
                # (hh-major, matching the kt_out tile order)
                for hh in range(2):
                    nc.gpsimd.dma_start(
                        out=v_sb[:, hh * 16 : (hh + 1) * 16, :],
                        in_=v_out[hh].rearrange("(j p) a -> p j a", p=128),
                    )

                # --- Q projection (overlaps the V exchanges); writes
                # directly into resident SBUF, no DRAM staging ---
                for qb in range(NQB):
                    for am in range(NA):
                        pq = p1pk.tile([128, 512], F32)
                        for dc in range(NC):
                            nc.tensor.matmul(
                                pq[:],
                                lhsT=wq[:, dc, am * 128 : (am + 1) * 128],
                                rhs=xs_all[:, qb, dc, :],
                                start=(dc == 0),
                                stop=(dc == NC - 1),
                            )
                        nc.scalar.activation(
                            qt_sb[:, am, qb * 512 : (qb + 1) * 512],
                            pq[:],
                            AF.Identity,
                            bias=bqc_sb[:, am : am + 1],
                        )

                nc.vector.memset(ones_k[:], 1.0)
                nc.vector.memset(ones_1[:], 1.0)

            # ---------------- Phase 2: attention ----------------
            with (
                tc.tile_pool(name="p2w", bufs=1) as p2w,
                tc.tile_pool(name="p2e", bufs=1) as p2e,
                tc.tile_pool(name="p2c", bufs=1) as p2c,
                tc.tile_pool(name="p2a", bufs=2) as p2a,
                tc.tile_pool(name="p2s", bufs=1) as p2s,
                tc.tile_pool(name="p2r", bufs=1) as p2r,
                tc.tile_pool(name="p2o", bufs=2) as p2o,
                tc.tile_pool(name="pps", bufs=2, space="PSUM") as pps,
                tc.tile_pool(name="ppsum", bufs=1, space="PSUM") as ppsum,
                tc.tile_pool(name="ppt", bufs=1, space="PSUM") as ppt,
                tc.tile_pool(name="ppc", bufs=2, space="PSUM") as ppc,
                tc.tile_pool(name="ppo", bufs=2, space="PSUM") as ppo,
            ):
                wo_sb = p2w.tile([128, NC, DIM], BF)     # WoT, 2.1 MB
                bob_sb = p2w.tile([128, DIM], BF)
                nc.sync.dma_start(out=wo_sb[:], in_=WoT[:])
                nc.sync.dma_start(out=bob_sb[:], in_=bob[:])

                # exp tiles live in a 48-slot ring (1.5 q-blocks): block qb's
                # k-tile kt sits at slot (32*qb + kt) % 48.  The pipeline
                # emits the next block's scores in two halves (ctx-A half
                # after ctxA, ctx-B half after do_out), so every ring
                # overwrite lands on slices whose reader already retired.
                et_ring = p2e.tile([128, 48, QB], BF, name="et_ring")

                def slot(qb, kt):
                    return (32 * qb + kt) % 48

                def do_scores_half(qb, hh, acc):
                    # scores^T + exp for pair-member hh (16 k-tiles); k-tile
                    # groups of 4 share one KT load.  Denominators accumulate
                    # on the vector engine alongside.
                    for sb in range(SB):
                            ks = p2k.tile([128, NC, 512], BF, name=f"ks{qb}_{hh}{sb}", tag="ks")
                            nc.sync.dma_start(
                                out=ks[:],
                                in_=kt_out[hh, sb],
                            )
                            ebase = hh * 16 + sb * 4
                            for kt4 in range(4):
                                ps = pps.tile([128, QB], F32, name=f"ps{qb}_{ebase+kt4}", tag="ps")
                                for ac in range(NC):
                                    nc.tensor.matmul(
                                        ps[:],
                                        lhsT=ks[:, ac, kt4 * 128 : (kt4 + 1) * 128],
                                        rhs=qt_sb[:, ac, qb * QB : (qb + 1) * QB],
                                        start=(ac == 0),
                                        stop=(ac == NC - 1),
                                    )
                                kt = ebase + kt4
                                nc.scalar.activation(
                                    et_ring[:, slot(qb, kt), :],
                                    ps[:],
                                    AF.Exp,
                                    scale=float(SCALE),
                                )
                                if kt == 0:
                                    nc.vector.tensor_copy(
                                        acc[:], et_ring[:, slot(qb, 0), :]
                                    )
                                else:
                                    nc.vector.tensor_add(
                                        acc[:], acc[:], et_ring[:, slot(qb, kt), :]
                                    )

                def do_rowsum(qb, acc):
                    # single partition-reduction matmul over the accumulated
                    # exp sums
                    p_row = ppsum.tile([1, QB], F32, name=f"p_row{qb}", tag="p_row")
                    nc.tensor.matmul(
                        p_row[:],
                        lhsT=ones_k[:, 0:1],
                        rhs=acc[:],
                        start=True,
                        stop=True,
                    )
                    srow = p2s.tile([1, QB], F32, name=f"srow{qb}", tag="srow")
                    nc.scalar.copy(srow[:], p_row[:])
                    return srow

                def do_recips(qb, srow):
                    recips = p2r.tile([128, 4], F32, name=f"recips{qb}", tag="recips")
                    for qi in range(4):
                        ptt = ppt.tile([128, 1], F32, name=f"ptt{qb}_{qi}", tag="ptt")
                        nc.tensor.matmul(
                            ptt[:],
                            lhsT=srow[0:1, qi * 128 : (qi + 1) * 128],
                            rhs=ones_1[0:1, 0:1],
                            start=True,
                            stop=True,
                        )
                        nc.vector.reciprocal(recips[:, qi : qi + 1], ptt[:])
                    return recips

                def do_ctxA(qb):
                    # first gather half of ctx^T
                    ct = p2c.tile([128, NA, QB], BF, name=f"ct{qb}", tag="ct")
                    for at in range(NA):
                        pc = ppc.tile([128, QB], F32, name=f"pcA{qb}_{at}", tag="pc")
                        for kt in range(NK // 2):
                            nc.tensor.matmul(
                                pc[:],
                                lhsT=v_sb[:, kt, at * 128 : (at + 1) * 128],
                                rhs=et_ring[:, slot(qb, kt), :],
                                start=(kt == 0),
                                stop=(kt == NK // 2 - 1),
                            )
                        nc.vector.tensor_copy(ct[:, at, :], pc[:])
                    return ct

                def do_ctxB(qb, ct):
                    for at in range(NA):
                        pc = ppc.tile([128, QB], F32, name=f"pcB{qb}_{at}", tag="pc")
                        for kt in range(NK // 2, NK):
                            nc.tensor.matmul(
                                pc[:],
                                lhsT=v_sb[:, kt, at * 128 : (at + 1) * 128],
                                rhs=et_ring[:, slot(qb, kt), :],
                                start=(kt == NK // 2),
                                stop=(kt == NK - 1),
                            )
                        nc.vector.tensor_add(ct[:, at, :], pc[:], ct[:, at, :])
                    return ct

                def do_out(qb, ct, recips):
                    # output projection + deferred softmax normalization + bias
                    for qi in range(4):
                        for half in range(2):
                            po = ppo.tile([128, 512], F32, name=f"po{qb}_{qi}{half}", tag="po")
                            for ac in range(NC):
                                nc.tensor.matmul(
                                    po[:],
                                    lhsT=ct[:, ac, qi * 128 : (qi + 1) * 128],
                                    rhs=wo_sb[:, ac, half * 512 : (half + 1) * 512],
                                    start=(ac == 0),
                                    stop=(ac == NC - 1),
                                )
                            ob = p2o.tile([128, 512], F32, name=f"ob{qb}_{qi}{half}", tag="ob")
                            nc.vector.tensor_scalar(
                                ob[:],
                                po[:],
                                recips[:, qi : qi + 1],
                                None,
                                op0=mybir.AluOpType.mult,
                            )
                            nc.vector.tensor_add(
                                ob[:], ob[:], bob_sb[:, half * 512 : (half + 1) * 512]
                            )
                            nc.sync.dma_start(
                                out=out[
                                    (qb * 4 + qi) * 128 : (qb * 4 + qi + 1) * 128,
                                    half * 512 : (half + 1) * 512,
                                ],
                                in_=ob[:],
                            )

                # software pipeline: the next block's scores are emitted in
                # two halves — ctx-A half between ctxA and ctxB, ctx-B half
                # after do_out — so the PE always has independent work while
                # the current block's ctx/out chain settles, and the et ring
                # overwrites only retired slices.  recips emission sits after
                # ctxA so the tiny transpose matmuls never head-of-line-block
                # the PE behind the scalar srow copy.
                acc = p2a.tile([128, QB], F32, name="acc0", tag="acc")
                do_scores_half(0, 0, acc)
                do_scores_half(0, 1, acc)
                acc_next = None
                for qb in range(NQB):
                    s = do_rowsum(qb, acc)
                    ct = do_ctxA(qb)
                    r = do_recips(qb, s)
                    if qb + 1 < NQB:
                        acc_next = p2a.tile([128, QB], F32, name=f"acc{qb+1}", tag="acc")
                        do_scores_half(qb + 1, 0, acc_next)
                    ct = do_ctxB(qb, ct)
                    do_out(qb, ct, r)
                    if qb + 1 < NQB:
                        do_scores_half(qb + 1, 1, acc_next)
                    acc = acc_next
            ksp.__exit__(None, None, None)

    _split_multiwaits(nc)
    return nc


_NC_CACHE = None


def _get_nc():
    global _NC_CACHE
    if _NC_CACHE is None:
        _NC_CACHE = _build()
    return _NC_CACHE


def kernel(x, Wq, bq, Wk, bk, Wv, bv, Wo, bo):
    global LAST_RESULT
    bf16 = ml_dtypes.bfloat16
    x = np.asarray(x, np.float32)

    def tile128(m):
        # [R, C] with R = 128*nc -> [128, nc, C], row r=c*128+p -> (p, c)
        R, C = m.shape
        return np.ascontiguousarray(
            m.reshape(R // 128, 128, C).transpose(1, 0, 2)
        )

    WqTt = tile128(np.asarray(Wq, np.float32).T.astype(bf16))
    WkTt = tile128(np.asarray(Wk, np.float32).T.astype(bf16))
    WvTt = tile128(np.asarray(Wv, np.float32).T.astype(bf16))
    WoTt = tile128(np.asarray(Wo, np.float32).T.astype(bf16))
    bqc = np.ascontiguousarray(np.asarray(bq, np.float32).reshape(NA, 128).T)
    bkc = np.ascontiguousarray(np.asarray(bk, np.float32).reshape(NA, 128).T)
    bvb = np.ascontiguousarray(np.broadcast_to(np.asarray(bv, np.float32), (128, A))).astype(bf16)
    bob = np.ascontiguousarray(np.broadcast_to(np.asarray(bo, np.float32), (128, DIM))).astype(bf16)

    in_maps = []
    for c in range(N_CORES):
        b, h = c // 2, c % 2
        xT = np.asarray(x[b, h * SQ : (h + 1) * SQ, :].T, np.float32).astype(bf16)
        # [DIM, SQ] -> [sb][128][c-chunk][512]
        xT4 = np.ascontiguousarray(
            xT.reshape(NC, 128, SB, 512).transpose(2, 1, 0, 3)
        )
        in_maps.append(
            {
                "xTq": xT4,
                "WqT": WqTt,
                "WkT": WkTt,
                "WvT": WvTt,
                "WoT": WoTt,
                "bqc": bqc,
                "bkc": bkc,
                "bvb": bvb,
                "bob": bob,
            }
        )

    nc = _get_nc()
    import os

    res = run_bass_kernel_spmd(
        nc,
        in_maps,
        core_ids=list(range(N_CORES)),
        trace=bool(os.environ.get("BASS_TRACE")),
    )
    LAST_RESULT = res

    out_full = np.empty((B, S, DIM), np.float32)
    for c in range(N_CORES):
        b, h = c // 2, c % 2
        out_full[b, h * SQ : (h + 1) * SQ, :] = res.results[c]["out"]
    return out_full
